# revision 8
# baseline (speedup 1.0000x reference)
"""Causal single-head attention (B=4, S=4096, D=768) on 8 TRN2 NeuronCores.

Sharding: core = (batch b = core//2, half h = core%2). Per batch, the 32
query blocks of 128 rows are split between the two cores in a
causally-balanced interleave: slot s (0..15) of core (b, h) handles query
rows [256*s + 128*h, 256*s + 128*h + 128).  Slots are grouped 4-at-a-time
(group t = slots 4t..4t+3, 512 query columns) and each group processes the
key window [0, 1024*(t+1)) -- identical program shape on every core; the
h-dependent causal boundary is handled by two data-driven [128,128]
multiplicative mask tiles (inputs), so a single NEFF runs SPMD on all 8
cores.

Precision/speed: all large GEMMs run as fp8e4m3 DoubleRow matmuls (PE
processes 2 rows/cycle, contraction 256 deep) on hi/lo residual pairs:
every operand X is stored as X_hi = e4m3(X), X_lo = e4m3(X - X_hi), and
each GEMM computes the three significant cross terms
(hi*hi + lo*hi + hi*lo), recovering ~fp16-class accuracy at 0.75x the
fp16 PE cost.  Wq/Wk are pre-scaled by 16 (folded back via the softmax
scale) so their lo-residuals stay out of the e4m3 subnormal range.
Scores are computed transposed, St[k, q], so after exp the P tiles
directly feed the P@x matmul lhsT-free.  No max-subtraction: scaled
scores are ~N(0,1) (max |z| ~ 6.5); exp is shifted by -2 so P <= e^4.5
fits e4m3 (max 240) and softmax shift-invariance cancels the shift.
The softmax denominator comes from near-free [128,1] ones-matmuls on the
resident P tiles.  The final (P@x)@Wv GEMM stays float32r for accuracy
(output-facing), as does the fp32 PSUM accumulation everywhere.
End-to-end max error vs the fp32 reference is ~6e-3 of absmax.
"""

import math

import numpy as np

B, S, D = 4, 4096, 768
P = 128
DT = D // P            # 6 d-chunks of 128
CP = DT // 2           # 3 chunk-pairs of 256 (DoubleRow contraction)
NK = S // P            # 32 key tiles
NG = 4                 # query groups per core
QG = 512               # query columns per group
NSLOT = 16             # 128-row query blocks per core
QW = NSLOT * P         # 2048 query rows per core
WS = 16.0              # Wq/Wk pre-scale (keeps e4m3 lo-residuals normal)
SCALE = 1.0 / (math.sqrt(D) * WS * WS)
SH = 2.0               # exp shift: P = exp(z - SH) <= e^4.5 < 240

F16 = np.float16

_CACHE = {}

# the 3 significant hi/lo cross terms of a product A*B, as (A-plane, B-plane)
_TERMS = ((0, 0), (1, 0), (0, 1))


def _build():
    import concourse.tile as tile
    from concourse import bacc, mybir

    f32 = mybir.dt.float32
    f32r = mybir.dt.float32r
    f16 = mybir.dt.float16
    e4 = mybir.dt.float8e4
    Exp = mybir.ActivationFunctionType.Exp
    Copy = mybir.ActivationFunctionType.Copy
    DR = mybir.MatmulPerfMode.DoubleRow

    nc = bacc.Bacc(
        "TRN2",
        target_bir_lowering=False,
        debug=False,
        enable_asserts=False,
        num_devices=8,
    )

    def din(name, shape, dt=e4):
        return nc.dram_tensor(name, shape, dt, kind="ExternalInput").ap()

    xt = [din("xt_hi", [D, S]), din("xt_lo", [D, S])]
    xq = [din("xq_hi", [D, QW]), din("xq_lo", [D, QW])]
    xn = [din("xn_hi", [S, D]), din("xn_lo", [S, D])]
    wq = [din("wq_hi", [D, D]), din("wq_lo", [D, D])]
    wk = [din("wk_hi", [D, D]), din("wk_lo", [D, D])]
    wv = din("wv", [D, D], f16)
    masks = din("masks", [2, P, P], f16)
    out = nc.dram_tensor("out", [QW, D], f16, kind="ExternalOutput").ap()

    with tile.TileContext(nc, pool_alloc_mode="queue") as tc:
        with (
            tc.tile_pool(name="resid", bufs=1) as resid,
            tc.tile_pool(name="psS", bufs=3, space="PSUM") as psS,
            tc.tile_pool(name="utp", bufs=4, space="PSUM") as utp,
        ):
            # residents: K^T/Q^T hi+lo (fp8), x natural hi+lo, Wv (f32r)
            kt = resid.tile([P, 2, DT, S], e4)        # [hl, dchunk, keys]
            qt = resid.tile([P, 2, DT, QW], e4)       # [hl, dchunk, queries]
            xnat = resid.tile([P, NK, 2, D], e4)      # [ktile, hl, d]
            wv_r = resid.tile([P, DT, D], f32r)
            mask_sb = resid.tile([P, 2, P], f16)
            ones4 = resid.tile([P, 1], e4)
            shb = resid.tile([P, 1], f32)

            for r in range(2):
                nc.sync.dma_start(mask_sb[:, r, :], masks[r, :, :])
            for kk in range(NK):
                for hl in range(2):
                    nc.sync.dma_start(
                        xnat[:, kk, hl, :], xn[hl][kk * P : (kk + 1) * P, :]
                    )
            nc.vector.memset(ones4[:], 1.0)
            nc.vector.memset(shb[:], -SH)

            def proj_mms(ps, w_sb, xch, do):
                n = len(_TERMS) * CP
                i = 0
                for cp in range(CP):
                    for xpl, wpl in _TERMS:
                        nc.tensor.matmul(
                            ps[:],
                            w_sb[:, wpl, 2 * cp : 2 * cp + 2, do * P : (do + 1) * P],
                            xch[:, xpl, 2 * cp : 2 * cp + 2, :],
                            start=(i == 0),
                            stop=(i == n - 1),
                            perf_mode=DR,
                        )
                        i += 1

            # ---------------- Phase 1: projections (3-term fp8 DR) --------
            with tc.tile_pool(name="wqp", bufs=1) as wqp, tc.tile_pool(
                name="xinq", bufs=3
            ) as xinq:
                wq_sb = wqp.tile([P, 2, DT, D], e4)
                for hl in range(2):
                    for di in range(DT):
                        nc.sync.dma_start(
                            wq_sb[:, hl, di, :], wq[hl][di * P : (di + 1) * P, :]
                        )
                for qc in range(QW // 512):
                    xch = xinq.tile([P, 2, DT, 512], e4, tag="xin")
                    for hl in range(2):
                        for di in range(DT):
                            nc.sync.dma_start(
                                xch[:, hl, di, :],
                                xq[hl][
                                    di * P : (di + 1) * P, qc * 512 : (qc + 1) * 512
                                ],
                            )
                    for do in range(DT):
                        ps = psS.tile([P, 512], f32, tag="ps")
                        proj_mms(ps, wq_sb, xch, do)
                        sl = qt[:, 0, do, qc * 512 : (qc + 1) * 512]
                        nc.scalar.activation(sl, ps[:], Copy)
                        nc.vector.tensor_sub(
                            qt[:, 1, do, qc * 512 : (qc + 1) * 512], ps[:], sl
                        )

            with tc.tile_pool(name="wkv", bufs=1) as wkv, tc.tile_pool(
                name="xink", bufs=3
            ) as xink:
                wk_sb = wkv.tile([P, 2, DT, D], e4, tag="wk")
                wv_sb = wkv.tile([P, DT, D], f16, tag="wv")
                for di in range(DT):
                    nc.sync.dma_start(wv_sb[:, di, :], wv[di * P : (di + 1) * P, :])
                    nc.vector.tensor_copy(wv_r[:, di, :], wv_sb[:, di, :])
                for hl in range(2):
                    for di in range(DT):
                        nc.sync.dma_start(
                            wk_sb[:, hl, di, :], wk[hl][di * P : (di + 1) * P, :]
                        )
                for kc in range(S // 512):
                    xch = xink.tile([P, 2, DT, 512], e4, tag="xin")
                    for hl in range(2):
                        for di in range(DT):
                            nc.sync.dma_start(
                                xch[:, hl, di, :],
                                xt[hl][
                                    di * P : (di + 1) * P, kc * 512 : (kc + 1) * 512
                                ],
                            )
                    for do in range(DT):
                        ps = psS.tile([P, 512], f32, tag="ps")
                        proj_mms(ps, wk_sb, xch, do)
                        sl = kt[:, 0, do, kc * 512 : (kc + 1) * 512]
                        nc.scalar.activation(sl, ps[:], Copy)
                        nc.vector.tensor_sub(
                            kt[:, 1, do, kc * 512 : (kc + 1) * 512], ps[:], sl
                        )

            # ------------- Phase 2: attention (3-term fp8 DR) -------------
            with (
                tc.tile_pool(name="ptp", bufs=17) as ptp,
                tc.tile_pool(name="p16p", bufs=4) as p16p,
                tc.tile_pool(name="utsb", bufs=8) as utsb,
                tc.tile_pool(name="outp", bufs=2) as outp,
                tc.tile_pool(name="small", bufs=4) as small,
            ):
                for t in range(NG):
                    win = 8 * t + 8
                    wp = win // 2
                    pts = []
                    c0s = []
                    ut_ps = [utp.tile([P, QG], f32, tag="ut", name=f"ut{_b}") for _b in range(3)]

                    def ut_sweep1(j):
                        c0 = c0s[j]
                        for ti, (xpl, ppl) in enumerate(_TERMS):
                            for db in range(3):
                                nc.tensor.matmul(
                                    ut_ps[db][:, c0:QG],
                                    xnat[:, 2 * j : 2 * j + 2, xpl,
                                         db * P : (db + 1) * P],
                                    pts[j][:, :, ppl, c0:QG],
                                    start=(j == 0 and ti == 0),
                                    stop=(j == wp - 1 and ti == 2),
                                    perf_mode=DR,
                                )

                    for j in range(wp):
                        jj = j - 4 * t
                        c0 = jj * P if jj >= 1 else 0
                        c0s.append(c0)
                        pt = ptp.tile([P, 2, 2, QG], e4, tag="pt")  # [par, hl, q]
                        for par in range(2):
                            k = 2 * j + par
                            ps = psS.tile([P, QG], f32, tag="ps")
                            n = len(_TERMS) * CP
                            i = 0
                            for cp in range(CP):
                                for qpl, kpl in _TERMS:
                                    nc.tensor.matmul(
                                        ps[:, c0:QG],
                                        kt[:, kpl, 2 * cp : 2 * cp + 2,
                                           k * P : (k + 1) * P],
                                        qt[:, qpl, 2 * cp : 2 * cp + 2,
                                           t * QG + c0 : (t + 1) * QG],
                                        start=(i == 0),
                                        stop=(i == n - 1),
                                        perf_mode=DR,
                                    )
                                    i += 1
                            p16 = p16p.tile([P, QG], f16, tag="p16")
                            nc.scalar.activation(
                                p16[:, c0:QG], ps[:, c0:QG], Exp,
                                bias=shb[:, 0:1], scale=SCALE,
                            )
                            if k >= 8 * t:
                                rel = (k - 8 * t) % 2
                                jj2 = (k - 8 * t) // 2
                                nc.vector.tensor_mul(
                                    p16[:, jj2 * P : (jj2 + 1) * P],
                                    p16[:, jj2 * P : (jj2 + 1) * P],
                                    mask_sb[:, rel, :],
                                )
                            hi = pt[:, par, 0, c0:QG]
                            nc.scalar.activation(hi, p16[:, c0:QG], Copy)
                            nc.vector.tensor_sub(
                                pt[:, par, 1, c0:QG], p16[:, c0:QG], hi
                            )
                        pts.append(pt)
                        # Ut sweep 1 runs one pair behind the score pipeline
                        # so the PE never waits on the exp/split chain.
                        if j > 0:
                            ut_sweep1(j - 1)
                    ut_sweep1(wp - 1)

                    ut_sb = []
                    for db in range(3):
                        u = utsb.tile([P, QG], f32r, tag="usb")
                        nc.vector.tensor_copy(u[:], ut_ps[db][:])
                        ut_sb.append(u)
                    ut_ps2 = [utp.tile([P, QG], f32, tag="ut", name=f"ut2{_b}") for _b in range(3)]
                    for j in range(wp):
                        c0 = c0s[j]
                        for ti, (xpl, ppl) in enumerate(_TERMS):
                            for db in range(3):
                                nc.tensor.matmul(
                                    ut_ps2[db][:, c0:QG],
                                    xnat[:, 2 * j : 2 * j + 2, xpl,
                                         (db + 3) * P : (db + 4) * P],
                                    pts[j][:, :, ppl, c0:QG],
                                    start=(j == 0 and ti == 0),
                                    stop=(j == wp - 1 and ti == 2),
                                    perf_mode=DR,
                                )
                    for db in range(3):
                        u = utsb.tile([P, QG], f32r, tag="usb")
                        nc.vector.tensor_copy(u[:], ut_ps2[db][:])
                        ut_sb.append(u)

                    # finals per 128-query block
                    for jb in range(4):
                        pso = utp.tile([P, 512], f32, tag="ut")
                        pso2f = utp.tile([P, 512], f32, tag="ut")
                        pso2 = pso2f[:, 0:256]
                        for di in range(DT):
                            nc.tensor.matmul(
                                pso[:],
                                ut_sb[di][:, jb * P : (jb + 1) * P],
                                wv_r[:, di, 0:512],
                                start=(di == 0),
                                stop=(di == DT - 1),
                            )
                        for di in range(DT):
                            nc.tensor.matmul(
                                pso2[:],
                                ut_sb[di][:, jb * P : (jb + 1) * P],
                                wv_r[:, di, 512:768],
                                start=(di == 0),
                                stop=(di == DT - 1),
                            )
                        nkj = 8 * t + 2 * jb + 2
                        pslf = utp.tile([P, 512], f32, tag="ut")
                        psl = pslf[:, 0:1]
                        i = 0
                        for k in range(nkj):
                            for ppl in range(2):
                                nc.tensor.matmul(
                                    psl[:],
                                    pts[k // 2][:, k % 2, ppl,
                                                jb * P : (jb + 1) * P],
                                    ones4[:, 0:1],
                                    start=(i == 0),
                                    stop=(i == 2 * nkj - 1),
                                )
                                i += 1
                        linv = small.tile([P, 1], f32, tag="linv")
                        nc.vector.reciprocal(linv[:], psl[:])
                        osb = outp.tile([P, D], f16, tag="osb")
                        nc.vector.tensor_scalar_mul(osb[:, 0:512], pso[:], linv[:])
                        nc.vector.tensor_scalar_mul(
                            osb[:, 512:768], pso2[:], linv[:]
                        )
                        s = 4 * t + jb
                        nc.sync.dma_start(out[s * P : (s + 1) * P, :], osb[:])

    nc.compile()
    return nc


def _get_nc():
    if "nc" not in _CACHE:
        _CACHE["nc"] = _build()
    return _CACHE["nc"]


def _hilo(a):
    import ml_dtypes

    E4 = ml_dtypes.float8_e4m3
    hi = a.astype(E4)
    lo = (a - hi.astype(np.float32)).astype(E4)
    return hi, lo


def _make_in_maps(x, Wq, Wk, Wv):
    x = np.asarray(x, dtype=np.float32)

    def wplanes(w):
        w16 = np.ascontiguousarray(np.asarray(w, np.float32) * WS).astype(F16)
        return _hilo(w16.astype(np.float32))

    wq_hi, wq_lo = wplanes(Wq)
    wk_hi, wk_lo = wplanes(Wk)
    wv = np.ascontiguousarray(np.asarray(Wv, dtype=np.float32)).astype(F16)

    tri = (np.arange(P)[:, None] <= np.arange(P)[None, :]).astype(np.float32)
    ones = np.ones((P, P), dtype=np.float32)
    zeros = np.zeros((P, P), dtype=np.float32)
    mask_h = [
        np.stack([tri, zeros]).astype(F16),  # h=0: rel0 tri, rel1 zero
        np.stack([ones, tri]).astype(F16),   # h=1: rel0 ones, rel1 tri
    ]

    # x is uploaded as the zero-copy [8*QW, D] fp16 reshape (each core's own
    # query rows); the fp8 hi/lo planes and layouts are derived on device.
    xsh = np.ascontiguousarray(x.astype(F16).reshape(8 * QW, D))
    in_maps = []
    for core in range(8):
        h = core % 2
        in_maps.append(
            {
                "xsh": xsh,  # global array, shared entry
                "wq_hi": wq_hi, "wq_lo": wq_lo,
                "wk_hi": wk_hi, "wk_lo": wk_lo,
                "wv": wv,
                "masks": mask_h[h],
            }
        )
    return in_maps


def _get_exec():
    """Build (once) a cached jitted SPMD callable over 8 cores.

    Mirrors concourse.bass2jax.run_bass_via_pjrt's multi-core path, but keeps
    the jitted function so repeat calls skip retracing.
    """
    if "exec" in _CACHE:
        return _CACHE["exec"]

    import jax
    import ml_dtypes
    from jax.sharding import Mesh, PartitionSpec
    from jax.experimental.shard_map import shard_map
    import concourse.mybir as mybir
    from concourse.bass2jax import (
        _bass_exec_p,
        install_neuronx_cc_hook,
        partition_id_tensor,
    )

    install_neuronx_cc_hook()
    nc = _get_nc()
    partition_name = nc.partition_id_tensor.name if nc.partition_id_tensor else None

    in_names, out_names, out_avals, zero_shapes = [], [], [], []
    for alloc in nc.m.functions[0].allocations:
        if not isinstance(alloc, mybir.MemoryLocationSet):
            continue
        name = alloc.memorylocations[0].name
        if alloc.kind == "ExternalInput":
            if name == partition_name:
                continue
            in_names.append(name)
        elif alloc.kind == "ExternalOutput":
            out_names.append(name)
            shape = tuple(alloc.tensor_shape)
            dtype = mybir.dt.np(alloc.dtype)
            out_avals.append(jax.core.ShapedArray(shape, dtype))
            zero_shapes.append((shape, dtype))
    n_params = len(in_names)
    n_outs = len(out_avals)
    all_names = in_names + out_names
    if partition_name is not None:
        all_names = all_names + [partition_name]
    donate = tuple(range(n_params, n_params + n_outs))

    def _body(*args):
        operands = list(args)
        if partition_name is not None:
            operands.append(partition_id_tensor())
        outs = _bass_exec_p.bind(
            *operands,
            out_avals=tuple(out_avals),
            in_names=tuple(all_names),
            out_names=tuple(out_names),
            lowering_input_output_aliases=(),
            sim_require_finite=True,
            sim_require_nnan=True,
            nc=nc,
        )
        return tuple(outs)

    devices = jax.devices()[:8]
    mesh = Mesh(np.asarray(devices), ("core",))
    # Weights are identical on every core: replicate instead of sharding so
    # they are uploaded once per call instead of 8x.
    replicated = {"wq_hi", "wq_lo", "wk_hi", "wk_lo", "wv"}
    in_specs = tuple(
        PartitionSpec() if name in replicated else PartitionSpec("core")
        for name in in_names
    ) + (PartitionSpec("core"),) * n_outs
    sharded = jax.jit(
        shard_map(
            _body,
            mesh=mesh,
            in_specs=in_specs,
            out_specs=(PartitionSpec("core"),) * n_outs,
            check_rep=False,
        ),
        donate_argnums=donate,
        keep_unused=True,
    )

    # On-device input prep (saves shipping 100MB/call): each core uploads only
    # its own 2048-row fp16 slice of x; a pairwise all_gather reconstructs the
    # batch's full [4096, 768] sequence, which is split into e4m3 hi/lo
    # residual planes and laid out as x^T / x^T-query-cols / x-natural --
    # all device-side.
    E4 = ml_dtypes.float8_e4m3

    def _prep_inputs(x_shard):
        import jax
        import jax.numpy as jnp
        from jax import lax

        h = lax.axis_index("core") % 2
        x_full = lax.all_gather(
            x_shard,
            "core",
            axis_index_groups=[[0, 1], [2, 3], [4, 5], [6, 7]],
            axis=0,
            tiled=True,
        )  # [S, D] fp16
        xf = x_full.astype(jnp.float32)
        # optimization_barrier: force the e4m3 casts to materialize exactly
        # once.  Without it XLA re-computes the cast per consumer with
        # DIFFERENT rounding (standalone cast is RNE, the cast fused into the
        # NKI transpose truncates), so hi + lo != x and the xt/xq/xn planes
        # disagree with each other.
        xhi = jax.lax.optimization_barrier(xf.astype(E4))
        xlo = jax.lax.optimization_barrier(
            (xf - xhi.astype(jnp.float32)).astype(E4)
        )
        xt_hi = jnp.transpose(xhi)  # [D, S]
        xt_lo = jnp.transpose(xlo)
        idx = jnp.arange(NSLOT) * 256 + h * P
        rows = (idx[:, None] + jnp.arange(P)[None, :]).reshape(-1)
        xq_hi = jnp.transpose(xhi[rows])  # [D, QW]
        xq_lo = jnp.transpose(xlo[rows])
        return xt_hi, xt_lo, xq_hi, xq_lo, xhi, xlo

    prep = jax.jit(
        shard_map(
            _prep_inputs,
            mesh=mesh,
            in_specs=(PartitionSpec("core"),),
            out_specs=(PartitionSpec("core"),) * 6,
            check_rep=False,
        )
    )
    _CACHE["exec"] = (
        sharded, in_names, out_names, out_avals, zero_shapes, replicated, prep, mesh,
    )
    return _CACHE["exec"]


def _concat_inputs(in_maps, in_names, replicated=None):
    if replicated is None:
        replicated = frozenset(("wq_hi", "wq_lo", "wk_hi", "wk_lo", "wv"))
    return [
        np.asarray(in_maps[0][name])
        if name in replicated
        else np.concatenate([np.asarray(m[name]) for m in in_maps], axis=0)
        for name in in_names
    ]


def _make_zeros(zero_shapes):
    return [
        np.zeros((8 * shape[0], *shape[1:]), dtype) for shape, dtype in zero_shapes
    ]


def _staged_inputs(prep, in_maps):
    xt_hi, xt_lo, xq_hi, xq_lo, xn_hi, xn_lo = prep(in_maps[0]["xsh"])
    return {
        "xt_hi": xt_hi, "xt_lo": xt_lo,
        "xq_hi": xq_hi, "xq_lo": xq_lo,
        "xn_hi": xn_hi, "xn_lo": xn_lo,
    }


def _run(in_maps):
    import jax

    (sharded, in_names, out_names, out_avals, zero_shapes, replicated,
     prep, mesh) = _get_exec()
    staged = _staged_inputs(prep, in_maps)
    concat_in = [
        staged[name] if name in staged
        else _concat_inputs(in_maps, [name], replicated)[0]
        for name in in_names
    ]
    # The kernel writes every output element, so the donated output buffers
    # never need zeroing; reuse the previous call's device-resident outputs
    # instead of shipping fresh zero arrays each call.
    donated = _CACHE.pop("outbuf", None)
    if donated is None:
        donated = _make_zeros(zero_shapes)
    out_arrs = sharded(*concat_in, *donated)
    _CACHE["outbuf"] = list(out_arrs)
    i = out_names.index("out")
    full = np.asarray(out_arrs[i]).reshape(8, *out_avals[i].shape)
    return [full[c] for c in range(8)]


def kernel(x, Wq, Wk, Wv):
    in_maps = _make_in_maps(x, Wq, Wk, Wv)
    outs = _run(in_maps)
    out = np.empty((B, S, D), dtype=np.float32)
    for core in range(8):
        b, h = core // 2, core % 2
        out[b].reshape(NSLOT, 2, P, D)[:, h] = outs[core].reshape(NSLOT, P, D)
    return out


# revision 14
# speedup vs baseline: 1.3227x; 1.3227x over previous
"""Causal single-head attention (B=4, S=4096, D=768) on 8 TRN2 NeuronCores.

Sharding: core = (batch b = core//2, half h = core%2). Per batch, the 32
query blocks of 128 rows are split between the two cores in a
causally-balanced interleave: slot s (0..15) of core (b, h) handles query
rows [256*s + 128*h, 256*s + 128*h + 128).  Slots are grouped 4-at-a-time
(group t = slots 4t..4t+3, 512 query columns) and each group processes the
key window [0, 1024*(t+1)) -- identical program shape on every core; the
h-dependent causal boundary is handled by two data-driven [128,128]
multiplicative mask tiles (inputs), so a single NEFF runs SPMD on all 8
cores.

Precision/speed: all large GEMMs run as fp8e4m3 DoubleRow matmuls (PE
processes 2 rows/cycle, contraction 256 deep) on hi/lo residual pairs:
every operand X is stored as X_hi = e4m3(X), X_lo = e4m3(X - X_hi), and
each GEMM computes the three significant cross terms
(hi*hi + lo*hi + hi*lo), recovering ~fp16-class accuracy at 0.75x the
fp16 PE cost.  Wq/Wk are pre-scaled by 16 (folded back via the softmax
scale) so their lo-residuals stay out of the e4m3 subnormal range.
Scores are computed transposed, St[k, q], so after exp the P tiles
directly feed the P@x matmul lhsT-free.  No max-subtraction: scaled
scores are ~N(0,1) (max |z| ~ 6.5); exp is shifted by -2 so P <= e^4.5
fits e4m3 (max 240) and softmax shift-invariance cancels the shift.
The softmax denominator comes from near-free [128,1] ones-matmuls on the
resident P tiles.  The final (P@x)@Wv GEMM stays float32r for accuracy
(output-facing), as does the fp32 PSUM accumulation everywhere.
End-to-end max error vs the fp32 reference is ~6e-3 of absmax.
"""

import math

import numpy as np

B, S, D = 4, 4096, 768
P = 128
DT = D // P            # 6 d-chunks of 128
CP = DT // 2           # 3 chunk-pairs of 256 (DoubleRow contraction)
NK = S // P            # 32 key tiles
NG = 4                 # query groups per core
QG = 512               # query columns per group
NSLOT = 16             # 128-row query blocks per core
QW = NSLOT * P         # 2048 query rows per core
WS = 16.0              # Wq/Wk pre-scale (keeps e4m3 lo-residuals normal)
SCALE = 1.0 / (math.sqrt(D) * WS * WS)
SH = 2.0               # exp shift: P = exp(z - SH) <= e^4.5 < 240

F16 = np.float16

_CACHE = {}

# the 3 significant hi/lo cross terms of a product A*B, as (A-plane, B-plane)
_TERMS = ((0, 0), (1, 0), (0, 1))


def _build():
    import concourse.tile as tile
    from concourse import bacc, mybir

    f32 = mybir.dt.float32
    f32r = mybir.dt.float32r
    f16 = mybir.dt.float16
    e4 = mybir.dt.float8e4
    Exp = mybir.ActivationFunctionType.Exp
    Copy = mybir.ActivationFunctionType.Copy
    DR = mybir.MatmulPerfMode.DoubleRow

    nc = bacc.Bacc(
        "TRN2",
        target_bir_lowering=False,
        debug=False,
        enable_asserts=False,
        num_devices=8,
    )

    def din(name, shape, dt=e4):
        return nc.dram_tensor(name, shape, dt, kind="ExternalInput").ap()

    xt = [din("xt_hi", [D, S]), din("xt_lo", [D, S])]
    xq = [din("xq_hi", [D, QW]), din("xq_lo", [D, QW])]
    xn = [din("xn_hi", [S, D]), din("xn_lo", [S, D])]
    wq = [din("wq_hi", [D, D]), din("wq_lo", [D, D])]
    wk = [din("wk_hi", [D, D]), din("wk_lo", [D, D])]
    wv = din("wv", [D, D], f16)
    masks = din("masks", [2, P, P], f16)
    out = nc.dram_tensor("out", [QW, D], f16, kind="ExternalOutput").ap()

    with tile.TileContext(nc, pool_alloc_mode="queue") as tc:
        with (
            tc.tile_pool(name="resid", bufs=1) as resid,
            tc.tile_pool(name="psS", bufs=3, space="PSUM") as psS,
            tc.tile_pool(name="utp", bufs=4, space="PSUM") as utp,
        ):
            # residents: K^T/Q^T hi+lo (fp8), x natural hi+lo, Wv (f32r)
            kt = resid.tile([P, 2, DT, S], e4)        # [hl, dchunk, keys]
            qt = resid.tile([P, 2, DT, QW], e4)       # [hl, dchunk, queries]
            xnat = resid.tile([P, NK, 2, D], e4)      # [ktile, hl, d]
            wv_r = resid.tile([P, DT, D], f32r)
            mask_sb = resid.tile([P, 2, P], f16)
            ones4 = resid.tile([P, 1], e4)
            shb = resid.tile([P, 1], f32)

            nc.sync.dma_start(mask_sb[:, :, :], masks.rearrange("r p c -> p r c"))
            nc.vector.memset(ones4[:], 1.0)
            nc.vector.memset(shb[:], -SH)

            def load_xnat(k0, ntile):
                # one DMA per plane for `ntile` consecutive key tiles:
                # dram rows 128*(k0+u)+p -> xnat[p, k0+u, hl, :]
                for hl in range(2):
                    nc.sync.dma_start(
                        xnat[:, k0 : k0 + ntile, hl, :],
                        xn[hl][k0 * P : (k0 + ntile) * P, :].rearrange(
                            "(u p) d -> p u d", p=P
                        ),
                    )

            def proj_mms(ps, w_sb, xch, do):
                n = len(_TERMS) * CP
                i = 0
                for cp in range(CP):
                    for xpl, wpl in _TERMS:
                        nc.tensor.matmul(
                            ps[:],
                            w_sb[:, wpl, 2 * cp : 2 * cp + 2, do * P : (do + 1) * P],
                            xch[:, xpl, 2 * cp : 2 * cp + 2, :],
                            start=(i == 0),
                            stop=(i == n - 1),
                            perf_mode=DR,
                        )
                        i += 1

            # ---------------- Phase 1: projections (3-term fp8 DR) --------
            with tc.tile_pool(name="wqp", bufs=1) as wqp, tc.tile_pool(
                name="xinq", bufs=3
            ) as xinq:
                wq_sb = wqp.tile([P, 2, DT, D], e4)
                for hl in range(2):
                    nc.sync.dma_start(
                        wq_sb[:, hl, :, :],
                        wq[hl].rearrange("(di p) d -> p di d", p=P),
                    )
                for qc in range(QW // 512):
                    xch = xinq.tile([P, 2, DT, 512], e4, tag="xin")
                    for hl in range(2):
                        nc.sync.dma_start(
                            xch[:, hl, :, :],
                            xq[hl][:, qc * 512 : (qc + 1) * 512].rearrange(
                                "(di p) c -> p di c", p=P
                            ),
                        )
                    for do in range(DT):
                        ps = psS.tile([P, 512], f32, tag="ps")
                        proj_mms(ps, wq_sb, xch, do)
                        sl = qt[:, 0, do, qc * 512 : (qc + 1) * 512]
                        nc.scalar.activation(sl, ps[:], Copy)
                        nc.vector.tensor_sub(
                            qt[:, 1, do, qc * 512 : (qc + 1) * 512], ps[:], sl
                        )

            with tc.tile_pool(name="wkv", bufs=1) as wkv, tc.tile_pool(
                name="xink", bufs=3
            ) as xink:
                wk_sb = wkv.tile([P, 2, DT, D], e4, tag="wk")
                wv_sb = wkv.tile([P, DT, D], f16, tag="wv")
                nc.sync.dma_start(
                    wv_sb[:, :, :], wv.rearrange("(di p) d -> p di d", p=P)
                )
                for di in range(DT):
                    nc.vector.tensor_copy(wv_r[:, di, :], wv_sb[:, di, :])
                for hl in range(2):
                    nc.sync.dma_start(
                        wk_sb[:, hl, :, :],
                        wk[hl].rearrange("(di p) d -> p di d", p=P),
                    )
                for kc in range(S // 512):
                    xch = xink.tile([P, 2, DT, 512], e4, tag="xin")
                    for hl in range(2):
                        nc.sync.dma_start(
                            xch[:, hl, :, :],
                            xt[hl][:, kc * 512 : (kc + 1) * 512].rearrange(
                                "(di p) c -> p di c", p=P
                            ),
                        )
                    # interleave the x-natural loads (needed first in phase 2)
                    # with the K-projection stream: one 4-tile DMA pair per
                    # 512-key chunk keeps HWDGE paced just behind the PE.
                    load_xnat(kc * 4, 4)
                    for do in range(DT):
                        ps = psS.tile([P, 512], f32, tag="ps")
                        proj_mms(ps, wk_sb, xch, do)
                        sl = kt[:, 0, do, kc * 512 : (kc + 1) * 512]
                        nc.scalar.activation(sl, ps[:], Copy)
                        nc.vector.tensor_sub(
                            kt[:, 1, do, kc * 512 : (kc + 1) * 512], ps[:], sl
                        )

            # ------------- Phase 2: attention (3-term fp8 DR) -------------
            with (
                tc.tile_pool(name="ptp", bufs=17) as ptp,
                tc.tile_pool(name="p16p", bufs=4) as p16p,
                tc.tile_pool(name="utsb", bufs=8) as utsb,
                tc.tile_pool(name="outp", bufs=2) as outp,
                tc.tile_pool(name="small", bufs=4) as small,
            ):
                for t in range(NG):
                    win = 8 * t + 8
                    wp = win // 2
                    pts = []
                    c0s = []
                    ut_ps = [utp.tile([P, QG], f32, tag="ut", name=f"ut{_b}") for _b in range(3)]

                    def ut_sweep1(j):
                        c0 = c0s[j]
                        for ti, (xpl, ppl) in enumerate(_TERMS):
                            for db in range(3):
                                nc.tensor.matmul(
                                    ut_ps[db][:, c0:QG],
                                    xnat[:, 2 * j : 2 * j + 2, xpl,
                                         db * P : (db + 1) * P],
                                    pts[j][:, :, ppl, c0:QG],
                                    start=(j == 0 and ti == 0),
                                    stop=(j == wp - 1 and ti == 2),
                                    perf_mode=DR,
                                )

                    for j in range(wp):
                        jj = j - 4 * t
                        c0 = jj * P if jj >= 1 else 0
                        c0s.append(c0)
                        pt = ptp.tile([P, 2, 2, QG], e4, tag="pt")  # [par, hl, q]
                        for par in range(2):
                            k = 2 * j + par
                            ps = psS.tile([P, QG], f32, tag="ps")
                            n = len(_TERMS) * CP
                            i = 0
                            for cp in range(CP):
                                for qpl, kpl in _TERMS:
                                    nc.tensor.matmul(
                                        ps[:, c0:QG],
                                        kt[:, kpl, 2 * cp : 2 * cp + 2,
                                           k * P : (k + 1) * P],
                                        qt[:, qpl, 2 * cp : 2 * cp + 2,
                                           t * QG + c0 : (t + 1) * QG],
                                        start=(i == 0),
                                        stop=(i == n - 1),
                                        perf_mode=DR,
                                    )
                                    i += 1
                            p16 = p16p.tile([P, QG], f16, tag="p16")
                            nc.scalar.activation(
                                p16[:, c0:QG], ps[:, c0:QG], Exp,
                                bias=shb[:, 0:1], scale=SCALE,
                            )
                            if k >= 8 * t:
                                rel = (k - 8 * t) % 2
                                jj2 = (k - 8 * t) // 2
                                nc.vector.tensor_mul(
                                    p16[:, jj2 * P : (jj2 + 1) * P],
                                    p16[:, jj2 * P : (jj2 + 1) * P],
                                    mask_sb[:, rel, :],
                                )
                            hi = pt[:, par, 0, c0:QG]
                            nc.scalar.activation(hi, p16[:, c0:QG], Copy)
                            nc.vector.tensor_sub(
                                pt[:, par, 1, c0:QG], p16[:, c0:QG], hi
                            )
                        pts.append(pt)
                        # Ut sweep 1 runs one pair behind the score pipeline
                        # so the PE never waits on the exp/split chain.
                        if j > 0:
                            ut_sweep1(j - 1)
                    ut_sweep1(wp - 1)

                    ut_sb = []
                    for db in range(3):
                        u = utsb.tile([P, QG], f32r, tag="usb")
                        nc.vector.tensor_copy(u[:], ut_ps[db][:])
                        ut_sb.append(u)
                    ut_ps2 = [utp.tile([P, QG], f32, tag="ut", name=f"ut2{_b}") for _b in range(3)]
                    for j in range(wp):
                        c0 = c0s[j]
                        for ti, (xpl, ppl) in enumerate(_TERMS):
                            for db in range(3):
                                nc.tensor.matmul(
                                    ut_ps2[db][:, c0:QG],
                                    xnat[:, 2 * j : 2 * j + 2, xpl,
                                         (db + 3) * P : (db + 4) * P],
                                    pts[j][:, :, ppl, c0:QG],
                                    start=(j == 0 and ti == 0),
                                    stop=(j == wp - 1 and ti == 2),
                                    perf_mode=DR,
                                )
                    for db in range(3):
                        u = utsb.tile([P, QG], f32r, tag="usb")
                        nc.vector.tensor_copy(u[:], ut_ps2[db][:])
                        ut_sb.append(u)

                    # finals per 128-query block
                    osb = outp.tile([P, 4, D], f16, tag="osb")
                    for jb in range(4):
                        pso = utp.tile([P, 512], f32, tag="ut")
                        pso2f = utp.tile([P, 512], f32, tag="ut")
                        pso2 = pso2f[:, 0:256]
                        for di in range(DT):
                            nc.tensor.matmul(
                                pso[:],
                                ut_sb[di][:, jb * P : (jb + 1) * P],
                                wv_r[:, di, 0:512],
                                start=(di == 0),
                                stop=(di == DT - 1),
                            )
                        for di in range(DT):
                            nc.tensor.matmul(
                                pso2[:],
                                ut_sb[di][:, jb * P : (jb + 1) * P],
                                wv_r[:, di, 512:768],
                                start=(di == 0),
                                stop=(di == DT - 1),
                            )
                        nkj = 8 * t + 2 * jb + 2
                        pslf = utp.tile([P, 512], f32, tag="ut")
                        psl = pslf[:, 0:1]
                        i = 0
                        for k in range(nkj):
                            for ppl in range(2):
                                nc.tensor.matmul(
                                    psl[:],
                                    pts[k // 2][:, k % 2, ppl,
                                                jb * P : (jb + 1) * P],
                                    ones4[:, 0:1],
                                    start=(i == 0),
                                    stop=(i == 2 * nkj - 1),
                                )
                                i += 1
                        linv = small.tile([P, 1], f32, tag="linv")
                        nc.vector.reciprocal(linv[:], psl[:])
                        nc.vector.tensor_scalar_mul(
                            osb[:, jb, 0:512], pso[:], linv[:]
                        )
                        nc.vector.tensor_scalar_mul(
                            osb[:, jb, 512:768], pso2[:], linv[:]
                        )
                    nc.sync.dma_start(
                        out[4 * t * P : (4 * t + 4) * P, :].rearrange(
                            "(jb p) d -> p jb d", p=P
                        ),
                        osb[:],
                    )

    nc.compile()
    return nc


def _get_nc():
    if "nc" not in _CACHE:
        _CACHE["nc"] = _build()
    return _CACHE["nc"]


def _hilo(a):
    import ml_dtypes

    E4 = ml_dtypes.float8_e4m3
    hi = a.astype(E4)
    lo = (a - hi.astype(np.float32)).astype(E4)
    return hi, lo


def _make_in_maps(x, Wq, Wk, Wv):
    x = np.asarray(x, dtype=np.float32)

    def wplanes(w):
        w16 = np.ascontiguousarray(np.asarray(w, np.float32) * WS).astype(F16)
        return _hilo(w16.astype(np.float32))

    wq_hi, wq_lo = wplanes(Wq)
    wk_hi, wk_lo = wplanes(Wk)
    wv = np.ascontiguousarray(np.asarray(Wv, dtype=np.float32)).astype(F16)

    tri = (np.arange(P)[:, None] <= np.arange(P)[None, :]).astype(np.float32)
    ones = np.ones((P, P), dtype=np.float32)
    zeros = np.zeros((P, P), dtype=np.float32)
    mask_h = [
        np.stack([tri, zeros]).astype(F16),  # h=0: rel0 tri, rel1 zero
        np.stack([ones, tri]).astype(F16),   # h=1: rel0 ones, rel1 tri
    ]

    # x is uploaded as the zero-copy [8*QW, D] fp16 reshape (each core's own
    # query rows); the fp8 hi/lo planes and layouts are derived on device.
    xsh = np.ascontiguousarray(x.astype(F16).reshape(8 * QW, D))
    in_maps = []
    for core in range(8):
        h = core % 2
        in_maps.append(
            {
                "xsh": xsh,  # global array, shared entry
                "wq_hi": wq_hi, "wq_lo": wq_lo,
                "wk_hi": wk_hi, "wk_lo": wk_lo,
                "wv": wv,
                "masks": mask_h[h],
            }
        )
    return in_maps


def _get_exec():
    """Build (once) a cached jitted SPMD callable over 8 cores.

    Mirrors concourse.bass2jax.run_bass_via_pjrt's multi-core path, but keeps
    the jitted function so repeat calls skip retracing.
    """
    if "exec" in _CACHE:
        return _CACHE["exec"]

    import jax
    import ml_dtypes
    from jax.sharding import Mesh, PartitionSpec
    from jax.experimental.shard_map import shard_map
    import concourse.mybir as mybir
    from concourse.bass2jax import (
        _bass_exec_p,
        install_neuronx_cc_hook,
        partition_id_tensor,
    )

    install_neuronx_cc_hook()
    nc = _get_nc()
    partition_name = nc.partition_id_tensor.name if nc.partition_id_tensor else None

    in_names, out_names, out_avals, zero_shapes = [], [], [], []
    for alloc in nc.m.functions[0].allocations:
        if not isinstance(alloc, mybir.MemoryLocationSet):
            continue
        name = alloc.memorylocations[0].name
        if alloc.kind == "ExternalInput":
            if name == partition_name:
                continue
            in_names.append(name)
        elif alloc.kind == "ExternalOutput":
            out_names.append(name)
            shape = tuple(alloc.tensor_shape)
            dtype = mybir.dt.np(alloc.dtype)
            out_avals.append(jax.core.ShapedArray(shape, dtype))
            zero_shapes.append((shape, dtype))
    n_params = len(in_names)
    n_outs = len(out_avals)
    all_names = in_names + out_names
    if partition_name is not None:
        all_names = all_names + [partition_name]
    donate = tuple(range(n_params, n_params + n_outs))

    def _body(*args):
        operands = list(args)
        if partition_name is not None:
            operands.append(partition_id_tensor())
        outs = _bass_exec_p.bind(
            *operands,
            out_avals=tuple(out_avals),
            in_names=tuple(all_names),
            out_names=tuple(out_names),
            lowering_input_output_aliases=(),
            sim_require_finite=True,
            sim_require_nnan=True,
            nc=nc,
        )
        return tuple(outs)

    devices = jax.devices()[:8]
    mesh = Mesh(np.asarray(devices), ("core",))
    # Weights are identical on every core: replicate instead of sharding so
    # they are uploaded once per call instead of 8x.
    replicated = {"wq_hi", "wq_lo", "wk_hi", "wk_lo", "wv"}
    in_specs = tuple(
        PartitionSpec() if name in replicated else PartitionSpec("core")
        for name in in_names
    ) + (PartitionSpec("core"),) * n_outs
    sharded = jax.jit(
        shard_map(
            _body,
            mesh=mesh,
            in_specs=in_specs,
            out_specs=(PartitionSpec("core"),) * n_outs,
            check_rep=False,
        ),
        donate_argnums=donate,
        keep_unused=True,
    )

    # On-device input prep (saves shipping 100MB/call): each core uploads only
    # its own 2048-row fp16 slice of x; a pairwise all_gather reconstructs the
    # batch's full [4096, 768] sequence, which is split into e4m3 hi/lo
    # residual planes and laid out as x^T / x^T-query-cols / x-natural --
    # all device-side.
    E4 = ml_dtypes.float8_e4m3

    def _prep_inputs(x_shard):
        import jax
        import jax.numpy as jnp
        from jax import lax

        h = lax.axis_index("core") % 2
        x_full = lax.all_gather(
            x_shard,
            "core",
            axis_index_groups=[[0, 1], [2, 3], [4, 5], [6, 7]],
            axis=0,
            tiled=True,
        )  # [S, D] fp16
        xf = x_full.astype(jnp.float32)
        # optimization_barrier: force the e4m3 casts to materialize exactly
        # once.  Without it XLA re-computes the cast per consumer with
        # DIFFERENT rounding (standalone cast is RNE, the cast fused into the
        # NKI transpose truncates), so hi + lo != x and the xt/xq/xn planes
        # disagree with each other.
        xhi = jax.lax.optimization_barrier(xf.astype(E4))
        xlo = jax.lax.optimization_barrier(
            (xf - xhi.astype(jnp.float32)).astype(E4)
        )
        xt_hi = jnp.transpose(xhi)  # [D, S]
        xt_lo = jnp.transpose(xlo)
        idx = jnp.arange(NSLOT) * 256 + h * P
        rows = (idx[:, None] + jnp.arange(P)[None, :]).reshape(-1)
        xq_hi = jnp.transpose(xhi[rows])  # [D, QW]
        xq_lo = jnp.transpose(xlo[rows])
        return xt_hi, xt_lo, xq_hi, xq_lo, xhi, xlo

    prep = jax.jit(
        shard_map(
            _prep_inputs,
            mesh=mesh,
            in_specs=(PartitionSpec("core"),),
            out_specs=(PartitionSpec("core"),) * 6,
            check_rep=False,
        )
    )
    _CACHE["exec"] = (
        sharded, in_names, out_names, out_avals, zero_shapes, replicated, prep, mesh,
    )
    return _CACHE["exec"]


def _concat_inputs(in_maps, in_names, replicated=None):
    if replicated is None:
        replicated = frozenset(("wq_hi", "wq_lo", "wk_hi", "wk_lo", "wv"))
    return [
        np.asarray(in_maps[0][name])
        if name in replicated
        else np.concatenate([np.asarray(m[name]) for m in in_maps], axis=0)
        for name in in_names
    ]


def _make_zeros(zero_shapes):
    return [
        np.zeros((8 * shape[0], *shape[1:]), dtype) for shape, dtype in zero_shapes
    ]


def _staged_inputs(prep, in_maps):
    xt_hi, xt_lo, xq_hi, xq_lo, xn_hi, xn_lo = prep(in_maps[0]["xsh"])
    return {
        "xt_hi": xt_hi, "xt_lo": xt_lo,
        "xq_hi": xq_hi, "xq_lo": xq_lo,
        "xn_hi": xn_hi, "xn_lo": xn_lo,
    }


def _run(in_maps):
    import jax

    (sharded, in_names, out_names, out_avals, zero_shapes, replicated,
     prep, mesh) = _get_exec()
    staged = _staged_inputs(prep, in_maps)
    concat_in = [
        staged[name] if name in staged
        else _concat_inputs(in_maps, [name], replicated)[0]
        for name in in_names
    ]
    # The kernel writes every output element, so the donated output buffers
    # never need zeroing; reuse the previous call's device-resident outputs
    # instead of shipping fresh zero arrays each call.
    donated = _CACHE.pop("outbuf", None)
    if donated is None:
        donated = _make_zeros(zero_shapes)
    out_arrs = sharded(*concat_in, *donated)
    _CACHE["outbuf"] = list(out_arrs)
    i = out_names.index("out")
    full = np.asarray(out_arrs[i]).reshape(8, *out_avals[i].shape)
    return [full[c] for c in range(8)]


def kernel(x, Wq, Wk, Wv):
    in_maps = _make_in_maps(x, Wq, Wk, Wv)
    outs = _run(in_maps)
    out = np.empty((B, S, D), dtype=np.float32)
    for core in range(8):
        b, h = core // 2, core % 2
        out[b].reshape(NSLOT, 2, P, D)[:, h] = outs[core].reshape(NSLOT, P, D)
    return out
